# revision 47
# baseline (speedup 1.0000x reference)
"""Trainium2 Bass kernel for nn_AllAmplitude (helicity-amplitude intensity).

Math: the reference contracts two spin-1 Wigner-D matrices per (resonance,
event) with a Breit-Wigner weight and sums |amp|^2 over external helicities.
Because D1 @ D2 = D^1(U1 U2) for the SU(2) elements U1, U2 of the two
rotations, the whole intensity collapses to

  I = 7 sum_r |w_r|^2
    + sum_{r<r'} 2 Re(w_r conj(w_r')) (10 Re(av)^2 + 2 Im(av)^2 - 3)

with av = conj(a_r') a_r + b_r' conj(b_r), (a, b) the Cayley-Klein parameters
of the composed rotation, and w_r the complex Breit-Wigner weight.

v3 changes vs v2:
 - Host packs the 7 inputs into 4 wave buffers already in SBUF layout
   (AG0 = alpha/gamma halves r01, M, B4 = betas, AG1 = r23), giving
   4KB-contiguous per-partition DMA rows (vs 512B descriptors) and a
   deterministic arrival order matched to the compute schedule. All input
   DMAs issue serially from Sync; first DVE op starts ~5us earlier.
 - Single ACT table set (trig_and_small: sin/square/copy); float biases
   (const-AP pool) instead of a memset pi2 tile; the f32->fp16 reciprocal
   cast moved from DVE to an ACT copy.
 - DVE stream reordered to be gap-free: A(h0), A(h1), BW-denominator,
   B(h0), B(h1), wre/wim, pairs.  GpSimd does nothing (concurrent Pool
   ops poison DVE throughput ~4x on this silicon - measured).

Sharding: pure data parallelism over the event axis N=262144 across the 8
NeuronCores (32768 events each, laid out [128 partitions x 256 events] with
the R=4 resonance slices side by side in the free axis).
"""

import numpy as np

import concourse.bass as bass
from concourse import bacc, mybir
from concourse.bass_utils import run_bass_kernel_spmd

F32 = mybir.dt.float32
BF16 = mybir.dt.bfloat16
FP16 = mybir.dt.float16
I16 = mybir.dt.int16
ALU = mybir.AluOpType
ACTF = mybir.ActivationFunctionType

R = 4
N_TOTAL = 262144
N_CORES = 8
N_CORE = N_TOTAL // N_CORES     # 32768 events per core
P = 128                         # SBUF partitions
E = N_CORE // P                 # 256 events per partition per resonance
W = R * E                       # 1024 free-dim of a full working tile
H = W // 2                      # half tile (r01 / r23)

MAGIC = float(np.float32(1.5 * 2.0**23))   # round-to-nearest-int bias trick
INV4PI = float(np.float32(1.0 / (4.0 * np.pi)))
TWOPI = float(np.float32(2.0 * np.pi))
HALFPI = float(np.float32(np.pi / 2.0))


def _register_custom_ops():
    import concourse.dve_ops as dve_ops
    from concourse.dve_spec import Spec, Src0, Src1, sq, lower, _has_src1
    from concourse.dve_uop import DveOpSpec
    from concourse.dve_ops import DveOp

    if any(op.name == "ANT_RANGE_RED_ADD" for op in dve_ops.OPS):
        return {op.name: op for op in dve_ops.OPS}

    def make_op(name, spec):
        shas = {}
        for ver in ("v3", "v4"):
            uops = lower(spec, ver=ver)
            shas[ver] = DveOpSpec(name=name, opcode=31, uops=uops,
                                  rd1_en=_has_src1(spec)).sha(ver)
        return DveOp(name, spec, subdim=False, uops_sha=shas)

    def _rr_ref(sgn):
        def ref(in0, in1, s0, s1, imm2):
            t = ((in0 + sgn * in1) * s0 + s1).astype(np.float32)
            r = ((t + imm2).astype(np.float32) - imm2).astype(np.float32)
            return (t - r).astype(np.float32)
        return ref

    from concourse.dve_spec import C0, C1, C2
    u = (Src0 + Src1) * C0 + C1
    rr_add = make_op("ANT_RANGE_RED_ADD",
                     Spec(body=u - ((u + C2) - C2), reference=_rr_ref(1.0)))
    u2 = (Src0 - Src1) * C0 + C1
    rr_sub = make_op("ANT_RANGE_RED_SUB",
                     Spec(body=u2 - ((u2 + C2) - C2), reference=_rr_ref(-1.0)))
    chi = make_op("ANT_CHI", Spec(
        body=sq(Src0) * C0 + sq(Src1) * C1 + C2,
        reference=lambda in0, in1, s0, s1, imm2:
            (in0 * in0 * s0 + in1 * in1 * s1 + imm2).astype(np.float32)))
    den = make_op("ANT_DEN", Spec(
        body=sq(C0 - Src0) + C1,
        reference=lambda in0, in1, s0, s1, imm2:
            ((s0 - in0) * (s0 - in0) + s1).astype(np.float32)))

    for op in (rr_add, rr_sub, chi, den):
        dve_ops.OPS.append(op)
        dve_ops._SUB_OPCODE_FOR_NAME[op.name] = (
            dve_ops._CUSTOM_DVE_ROW_BASE + len(dve_ops.OPS) - 1)
        dve_ops.CUSTOM_DVE_SPECS[op.name] = op.spec
    assert max(dve_ops._SUB_OPCODE_FOR_NAME.values()) < 0x20
    return {op.name: op for op in dve_ops.OPS}


def _rs(r):
    return slice(r * E, (r + 1) * E)


def build(m0, g0, coef_r, coef_i):
    OPS = _register_custom_ops()
    RR_ADD, RR_SUB, CHI, DEN = (OPS["ANT_RANGE_RED_ADD"], OPS["ANT_RANGE_RED_SUB"],
                                OPS["ANT_CHI"], OPS["ANT_DEN"])
    AT = FP16

    nc = bacc.Bacc("TRN2", target_bir_lowering=False, debug=False,
                   num_devices=N_CORES)
    # packed input waves (host-side layout, see kernel())
    a0_in = nc.dram_tensor("a0", (P, H), FP16, kind="ExternalInput").ap()
    a0b_in = nc.dram_tensor("a0b", (P, H), FP16, kind="ExternalInput").ap()
    g0_in = nc.dram_tensor("g0w", (P, 2 * H), FP16, kind="ExternalInput").ap()
    m_in = nc.dram_tensor("m4", (P, W), F32, kind="ExternalInput").ap()
    b4_in = nc.dram_tensor("b4", (P, 4 * H), FP16, kind="ExternalInput").ap()
    ag1_in = nc.dram_tensor("ag1", (P, 4 * H), FP16, kind="ExternalInput").ap()
    out_ap = nc.dram_tensor("out", (N_CORE,), F32, kind="ExternalOutput").ap()

    f32 = np.float32
    m0 = m0.astype(np.float64); g0 = g0.astype(np.float64)
    cR = [float(f32(coef_r[r] * np.cos(coef_i[r]))) for r in range(R)]
    cI = [float(f32(coef_r[r] * np.sin(coef_i[r]))) for r in range(R)]
    m0sq = [float(f32(m0[r] * m0[r])) for r in range(R)]
    y = [float(f32(m0[r] * g0[r])) for r in range(R)]
    ysq = [float(f32(f32(y[r]) * f32(y[r]))) for r in range(R)]
    k1 = [float(f32(-f32(cI[r]) * f32(y[r]))) for r in range(R)]
    k2 = [float(f32(f32(cR[r]) * f32(y[r]))) for r in range(R)]
    c27 = [float(f32(7.0 * (f32(cR[r])**2 + f32(cI[r])**2))) for r in range(R)]

    # ---- static SBUF allocation ----
    alloc = []
    def sb(name, shape, dt=F32):
        t = nc.alloc_sbuf_tensor(name, list(shape), dt)
        alloc.append(t)
        return t.ap()

    AG = [sb("ag0t", [P, 4 * H], FP16), sb("ag1t", [P, 4 * H], FP16)]
    B4 = sb("b4t", [P, 4 * H], FP16)      # [b1h0|b2h0|b1h1|b2h1]
    M4 = sb("m4t", [P, W])                # f32 m
    pi2 = sb("pi2", [P, 1])
    dmy = sb("dmy", [P, 1], AT)
    UVWZ = sb("UVWZ", [P, 4 * W], AT)     # [ub|wb|vb|zb]
    ub = UVWZ[:, 0:W]; wb = UVWZ[:, W:2*W]
    vb = UVWZ[:, 2*W:3*W]; zb = UVWZ[:, 3*W:4*W]
    # packed fracs [fA|fB|fC|fD] and their absolute values, fp16
    f4 = sb("f4", [P, 4 * W], AT)
    a4 = sb("a4", [P, 4 * W], AT)
    fA = f4[:, 0:W]; fB = f4[:, W:2*W]; fC = f4[:, 2*W:3*W]; fD = f4[:, 3*W:4*W]
    # sin/cos of composite angles: [As|Bs|Cs|Ds|Ac|Bc|Cc|Dc]
    SCT8 = sb("SCT8", [P, 8 * W], AT)
    # beta trig packed [cb1|sb1|cb2|sb2]
    BP = sb("BP", [P, 4 * W], AT)
    # Wigner magnitudes [M0|M1|M3|M2] ( = [ms_ab | ms_cd] )
    MS4 = sb("MS4", [P, 4 * W], AT)
    # pq products [psab0|psab1|pscd0|pscd1|pcab0|pcab1|pccd0|pccd1]
    PQ8 = sb("PQ8", [P, 8 * W], AT)
    AB4 = sb("AB4", [P, 4 * W], AT)   # [are|bim|aim|bre]
    msq = sb("msq", [P, W]); den = sb("den", [P, W]); rcp = den
    wp1 = sb("wp1", [P, W], AT); wp2 = sb("wp2", [P, W], AT)
    rcph = sb("rcph", [P, W], AT)
    WH = sb("WH", [P, 2 * W], BF16)   # [wreh|wimh]
    wreh = WH[:, 0:W]; wimh = WH[:, W:2*W]
    dall = sb("dall", [P, W], AT); dh = sb("dh", [P, 2 * E], AT)
    dg = sb("dg", [P, E], AT)
    acc = sb("acc", [P, E])
    # pair scratch (reused across the 3 shift groups; DVE program order)
    NP3 = 3 * E
    PT = sb("PT", [P, 4 * NP3], AT)    # 4 packed products
    SD = sb("SD", [P, 4 * NP3], AT)    # [S1ch0|S1ch1|DTch0|DTch1]
    DT = SD[:, 2*NP3:4*NP3]
    QT = sb("QT", [P, 4 * NP3], AT)    # [q1|q4|q2|q3]
    NP6 = 6 * E
    AVI2 = sb("AVI2", [P, 2 * NP6], AT)   # [avr|avi]
    avr = AVI2[:, 0:NP6]; avi = AVI2[:, NP6:2*NP6]
    chis = sb("chis", [P, NP6], BF16); gw = sb("gw", [P, NP6], BF16)
    GT6 = sb("GT6", [P, 2 * NP6], BF16)
    term = sb("term", [P, NP6], BF16)

    sem_w = [nc.alloc_semaphore(f"s_w{i}") for i in range(6)]
    with (
        nc.semaphore("s_out") as sem_out,
        nc.semaphore("act_sem") as act_sem,
        nc.semaphore("vec_sem") as vec_sem,
        nc.semaphore("g_sem") as g_sem,
        nc.Block() as block,
    ):
        # ------------- GPSIMD: pi/2 bias tile + parallel first-wave issue ---
        # (issue-only work; Pool compute would poison concurrent DVE ops,
        #  but these finish ~2us before the first DVE op starts)
        @block.gpsimd
        def _(gpsimd):
            nc.gpsimd.memset(pi2[:], HALFPI).then_inc(g_sem, 1)
            gpsimd.dma_start(AG[0][:, H:2*H], a0b_in[:]).then_inc(sem_w[5], 16)
            gpsimd.dma_start(AG[0][:, 2*H:4*H], g0_in[:]).then_inc(sem_w[1], 16)
        # ------------- SYNC: all input DMA issue, first output half ---------
        @block.sync
        def _(sync):
            sync.dma_start(AG[0][:, 0:H], a0_in[:]).then_inc(sem_w[0], 16)
            sync.dma_start(AG[1][:], ag1_in[:]).then_inc(sem_w[4], 16)
            sync.dma_start(M4[:], m_in[:]).then_inc(sem_w[2], 16)
            sync.dma_start(B4[:], b4_in[:]).then_inc(sem_w[3], 16)
            # (issue order = HBM service order; m before b4 so msq lands
            #  well ahead of the DVE den stage at either clock state)
            outv = out_ap.rearrange("(p e) -> p e", p=P, e=E)
            sync.wait_ge(vec_sem, 8)
            sync.dma_start(outv[:, 0:E//2], acc[:, 0:E//2]).then_inc(sem_out, 16)
            sync.wait_ge(sem_out, 32)

        # ------------- SCALAR (ACT) ----------------------------------------
        # act_sem: 1 dummy (forces the trig table load early), 2 msq,
        #          h0 sins: 3 cbs, 4 sbs, 5 AsBs, 6 CsDs, 7 cos4,
        #          h1 sins: 8-12, 13 rcph, 14-17 wp1, 18-21 wp2, 22-25 dall
        @block.scalar
        def _(scalar):
            bpv = BP.rearrange("p (c w) -> p c w", c=4, w=W)       # cb1 sb1 cb2 sb2
            b4v = B4.rearrange("p (c w) -> p c w", c=4, w=H)       # b1h0 b2h0 b1h1 b2h1
            sct8v = SCT8.rearrange("p (c w) -> p c w", c=8, w=W)
            f4v = f4.rearrange("p (c w) -> p c w", c=4, w=W)
            a4v = a4.rearrange("p (c w) -> p c w", c=4, w=W)
            scalar.wait_ge(g_sem, 1)
            # dummy 1-element Sin: hoists the trig ACT-table load to ~6us,
            # before any input data arrives (Square/Copy share this table)
            scalar.activation(dmy[:], pi2[:], ACTF.Sin).then_inc(act_sem, 1)  # 1
            scalar.wait_ge(sem_w[2], 16)
            scalar.activation(msq[:], M4[:], ACTF.Square).then_inc(act_sem, 1)  # 2
            scalar.wait_ge(sem_w[3], 16)
            for h in range(2):
                s = slice(h * H, h * H + H)
                # cos(b/2), sin(b/2) for both chains, this half
                scalar.activation(bpv[:, 0::2, s], b4v[:, 2*h:2*h+2, :],
                                  ACTF.Sin, scale=0.5,
                                  bias=pi2[:]).then_inc(act_sem, 1)  # cbs
                scalar.activation(bpv[:, 1::2, s], b4v[:, 2*h:2*h+2, :],
                                  ACTF.Sin, scale=0.5).then_inc(act_sem, 1)  # sbs
                scalar.wait_ge(vec_sem, 1 + 3 * h)    # fA,fB half h
                scalar.activation(sct8v[:, 0:2, s], f4v[:, 0:2, s], ACTF.Sin,
                                  scale=TWOPI).then_inc(act_sem, 1)   # AsBs
                scalar.wait_ge(vec_sem, 2 + 3 * h)    # fC,fD half h
                scalar.activation(sct8v[:, 2:4, s], f4v[:, 2:4, s], ACTF.Sin,
                                  scale=-TWOPI).then_inc(act_sem, 1)  # CsDs
                scalar.wait_ge(vec_sem, 3 + 3 * h)    # abs half h (all four)
                scalar.activation(sct8v[:, 4:8, s], a4v[:, :, s], ACTF.Sin,
                                  scale=-TWOPI, bias=pi2[:]).then_inc(act_sem, 1)
            scalar.wait_ge(vec_sem, 7)   # rcp
            scalar.activation(rcph[:], rcp[:], ACTF.Copy).then_inc(act_sem, 1)  # 13
            for r in range(R):
                scalar.activation(wp1[:, _rs(r)], msq[:, _rs(r)], ACTF.Copy,
                                  scale=-cR[r],
                                  bias=float(f32(cR[r]*m0sq[r] + k1[r]))
                                  ).then_inc(act_sem, 1)               # 14..17
            for r in range(R):
                scalar.activation(wp2[:, _rs(r)], msq[:, _rs(r)], ACTF.Copy,
                                  scale=-cI[r],
                                  bias=float(f32(cI[r]*m0sq[r] + k2[r]))
                                  ).then_inc(act_sem, 1)               # 18..21
            for r in range(R):
                scalar.activation(dall[:, _rs(r)], rcph[:, _rs(r)], ACTF.Copy,
                                  scale=c27[r]).then_inc(act_sem, 1)   # 22..25
            outv2 = out_ap.rearrange("(p e) -> p e", p=P, e=E)
            scalar.wait_ge(vec_sem, 9)
            scalar.dma_start(outv2[:, E//2:], acc[:, E//2:]).then_inc(sem_out, 16)

        # ------------- VECTOR (DVE) -----------------------------------------
        # vec_sem: per half h: 1+3h fAB, 2+3h fCD, 3+3h abs; 7 rcp;
        #          8/9 acc halves
        @block.vector
        def _(vector):
            # ---- stage A: per-half pre-adds + RR + packed abs ----
            agv = [AG[h].rearrange("p (c w) -> p c w", c=4, w=H) for h in range(2)]
            f4i = f4.bitcast(I16).rearrange("p (c w) -> p c w", c=4, w=W)
            a4i = a4.bitcast(I16).rearrange("p (c w) -> p c w", c=4, w=W)
            uvwz_v = UVWZ.rearrange("p (c w) -> p c w", c=4, w=W)
            for h in range(2):
                s = slice(h * H, h * H + H)
                if h == 0:
                    vector.wait_ge(sem_w[0], 16)   # alpha1 h0
                    vector.wait_ge(sem_w[5], 16)   # alpha2 h0
                    nc.vector.tensor_add(ub[:, s], agv[0][:, 0, :], agv[0][:, 1, :])
                    nc.vector.tensor_sub(vb[:, s], agv[0][:, 0, :], agv[0][:, 1, :])
                    vector.wait_ge(sem_w[1], 16)   # gammas h0
                    nc.vector.tensor_add(wb[:, s], agv[0][:, 2, :], agv[0][:, 3, :])
                    nc.vector.tensor_sub(zb[:, s], agv[0][:, 2, :], agv[0][:, 3, :])
                else:
                    vector.wait_ge(sem_w[4], 16)   # AG1
                    # [ub|wb] = [a1|g1] + [a2|g2]; [vb|zb] = [a1|g1] - [a2|g2]
                    nc.vector.tensor_add(uvwz_v[:, 0:2, s], agv[1][:, 0::2, :],
                                         agv[1][:, 1::2, :])
                    nc.vector.tensor_sub(uvwz_v[:, 2:4, s], agv[1][:, 0::2, :],
                                         agv[1][:, 1::2, :])
                nc.vector._custom_dve(RR_ADD, out=fA[:, s], in0=ub[:, s],
                                      in1=wb[:, s], s0=INV4PI, s1=0.0, imm2=MAGIC)
                nc.vector._custom_dve(RR_SUB, out=fB[:, s], in0=vb[:, s],
                                      in1=zb[:, s], s0=INV4PI, s1=0.0,
                                      imm2=MAGIC).then_inc(vec_sem, 1)  # 1/4
                nc.vector._custom_dve(RR_ADD, out=fC[:, s], in0=ub[:, s],
                                      in1=zb[:, s], s0=INV4PI, s1=0.0, imm2=MAGIC)
                nc.vector._custom_dve(RR_SUB, out=fD[:, s], in0=vb[:, s],
                                      in1=wb[:, s], s0=INV4PI, s1=0.0,
                                      imm2=MAGIC).then_inc(vec_sem, 1)  # 2/5
                nc.vector.tensor_scalar(a4i[:, :, s], f4i[:, :, s], 0x7FFF, None,
                                        ALU.bitwise_and).then_inc(vec_sem, 1)  # 3/6

            # ---- Breit-Wigner denominator (fills the pre-B ACT window) ----
            vector.wait_ge(act_sem, 2)   # msq
            for r in range(R):
                nc.vector._custom_dve(DEN, out=den[:, _rs(r)], in0=msq[:, _rs(r)],
                                      s0=m0sq[r], s1=ysq[r])
            nc.vector.reciprocal_approx_fast(out=rcp[:], in_=den[:]) \
                .then_inc(vec_sem, 1)   # 7

            # ---- stage B: Wigner magnitudes + merged 4-ch pq products ----
            ms4v = MS4.rearrange("p (c w) -> p c w", c=4, w=W)
            ms_ab = MS4[:, 0:2*W].rearrange("p (c w) -> p c w", c=2, w=W)
            ms_cd = MS4[:, 2*W:4*W].rearrange("p (c w) -> p c w", c=2, w=W)
            b1v = BP[:, 0:2*W].rearrange("p (c w) -> p c w", c=2, w=W)
            b2v = BP[:, 2*W:4*W].rearrange("p (c w) -> p c w", c=2, w=W)
            sct8vv = SCT8.rearrange("p (c w) -> p c w", c=8, w=W)
            pq8v = PQ8.rearrange("p (c w) -> p c w", c=8, w=W)
            ab4v = AB4.rearrange("p (c w) -> p c w", c=4, w=W)
            for h in range(2):
                s = slice(h * H, h * H + H)
                vector.wait_ge(act_sem, 4 + 5 * h)   # cbs+sbs half h
                nc.vector.tensor_mul(ms_ab[:, :, s], b1v[:, :, s], b2v[:, :, s])
                nc.vector.tensor_mul(ms_cd[:, :, s], b1v[:, :, s], b2v[:, 1::-1, s])
                vector.wait_ge(act_sem, 6 + 5 * h)   # AsBs+CsDs half h
                nc.vector.tensor_mul(pq8v[:, 0:4, s], ms4v[:, :, s],
                                     sct8vv[:, 0:4, s])
                vector.wait_ge(act_sem, 7 + 5 * h)   # cos4 half h
                nc.vector.tensor_mul(pq8v[:, 4:8, s], ms4v[:, :, s],
                                     sct8vv[:, 4:8, s])
                # AB4 = [are|bim|aim|bre]
                # [are|aim] = [Ac-prod - Bc-prod | Bs-prod - As-prod]
                nc.vector.tensor_sub(ab4v[:, 0::2, s], pq8v[:, 4::-3, s],
                                     pq8v[:, 5::-5, s])
                # [bim|bre] = [Cs-prod + Ds-prod | Cc-prod + Dc-prod]
                nc.vector.tensor_add(ab4v[:, 1::2, s], pq8v[:, 2::4, s],
                                     pq8v[:, 3::4, s])

            # ---- stage D: pairs (angle part; independent of the BW chain) ----
            gt6v = GT6.rearrange("p (c w) -> p c w", c=2, w=NP6)
            whv = WH.rearrange("p (c w) -> p c w", c=2, w=W)
            goffs = {1: 0, 2: 3 * E, 3: 5 * E}
            for sig in (1, 2, 3):
                n = (R - sig) * E
                go = goffs[sig]
                L = slice(0, n)
                Rr = slice(sig * E, sig * E + n)
                ptv = PT.rearrange("p (c w) -> p c w", c=4, w=NP3)
                sd4v = SD.rearrange("p (c w) -> p c w", c=4, w=NP3)
                av2v = AVI2.rearrange("p (c w) -> p c w", c=2, w=NP6)
                # ptv channels: [p_are|p_bim|p_aim|p_bre]
                nc.vector.tensor_mul(ptv[:, :, :n], ab4v[:, :, L], ab4v[:, :, Rr])
                # sd = [p_are+p_bre | p_aim+p_bim]
                nc.vector.tensor_add(sd4v[:, 0:2, :n],
                                     ptv[:, 0::2, :n], ptv[:, 3::-2, :n])
                qtv = QT.rearrange("p (c w) -> p c w", c=4, w=NP3)
                # one 4-ch op: [q1|q4|q2|q3] = [are|bim|aim|bre][L] *
                #              [aim|bre|are|bim][R] (group-reversed view)
                abgv = AB4.rearrange("p (g c w) -> p g c w", g=2, c=2, w=W)
                nc.vector.tensor_mul(
                    qtv[:, :, :n].rearrange("p (g c) w -> p g c w", g=2, c=2),
                    abgv[:, :, :, L], abgv[:, 1::-1, :, Rr])
                nc.vector.tensor_sub(sd4v[:, 2:4, :n],
                                     qtv[:, 0:2, :n], qtv[:, 2:4, :n])
                # [avr|avi] in one 2-ch add: ch0 = S1c0+S1c1, ch1 = DTc0+DTc1
                nc.vector.tensor_add(av2v[:, :, go:go+n],
                                     sd4v[:, 0::2, :n], sd4v[:, 1::2, :n])

            # ---- BW weights + pair weight products (late: ACT has slack) ----
            vector.wait_ge(act_sem, 21)   # wp1/wp2 + rcph done
            nc.vector.tensor_mul(wreh[:], wp1[:], rcph[:])
            nc.vector.tensor_mul(wimh[:], wp2[:], rcph[:])
            for sig in (1, 2, 3):
                n = (R - sig) * E
                go = goffs[sig]
                L = slice(0, n)
                Rr = slice(sig * E, sig * E + n)
                nc.vector.tensor_mul(gt6v[:, :, go:go+n], whv[:, :, L], whv[:, :, Rr])
            # diagonal (off the critical tail: only needs dall)
            vector.wait_ge(act_sem, 25)   # dall
            nc.vector.tensor_add(dh[:], dall[:, 0:2*E], dall[:, 2*E:4*E])
            nc.vector.tensor_add(dg[:], dh[:, 0:E], dh[:, E:2*E])
            nc.vector._custom_dve(CHI, out=chis[:], in0=avr[:], in1=avi[:],
                                  s0=20.0, s1=4.0, imm2=-6.0)
            nc.vector.tensor_add(gw[:], GT6[:, 0:NP6], GT6[:, NP6:2*NP6])
            nc.vector.tensor_mul(term[:], chis[:], gw[:])
            # tree-sum the 6 pair blocks: one packed 3-pair add, then merge
            tv = term.rearrange("p (c w) -> p c w", c=6, w=E)
            s3 = DT.rearrange("p (c w) -> p c w", c=2, w=NP3)  # reuse DT scratch
            nc.vector.tensor_tensor(s3[:, 0, 0:3*E].rearrange("p (c w) -> p c w", c=3, w=E),
                                    tv[:, 0::2, :], tv[:, 1::2, :], ALU.add)
            nc.vector.tensor_add(DT[:, 3*E:4*E], DT[:, 0:E], DT[:, E:2*E])
            nc.vector.tensor_add(dh[:, 0:E], DT[:, 2*E:3*E], DT[:, 3*E:4*E])
            # final: acc = pairs + diagonal, split for output DMA overlap
            nc.vector.tensor_add(acc[:, 0:E//2], dh[:, 0:E//2], dg[:, 0:E//2]) \
                .then_inc(vec_sem, 1)   # 8
            nc.vector.tensor_add(acc[:, E//2:E], dh[:, E//2:E], dg[:, E//2:E]) \
                .then_inc(vec_sem, 1)   # 9

    nc.compile()
    return nc


_CACHE = {}


def kernel(alpha1, beta1, gamma1, alpha2, beta2, gamma2, m, m0, g0,
           coef_r, coef_i, _want_trace=False):
    key = (np.asarray(m0, np.float32).tobytes(), np.asarray(g0, np.float32).tobytes(),
           np.asarray(coef_r, np.float32).tobytes(), np.asarray(coef_i, np.float32).tobytes())
    if key not in _CACHE:
        _CACHE[key] = build(np.asarray(m0, np.float32), np.asarray(g0, np.float32),
                            np.asarray(coef_r, np.float32), np.asarray(coef_i, np.float32))
    nc = _CACHE[key]

    f16 = np.float16
    a1 = np.asarray(alpha1, np.float32); a2 = np.asarray(alpha2, np.float32)
    g1 = np.asarray(gamma1, np.float32); g2 = np.asarray(gamma2, np.float32)
    b1 = np.asarray(beta1, np.float32); b2 = np.asarray(beta2, np.float32)
    mf = np.asarray(m, np.float32)

    in_maps = []
    for i in range(N_CORES):
        sl = slice(i * N_CORE, (i + 1) * N_CORE)
        # (R, N_CORE) -> (P, R, E): partition-major event blocks
        def prep(x):
            return x[:, sl].reshape(R, P, E).transpose(1, 0, 2)
        ag = np.stack([prep(a1), prep(a2), prep(g1), prep(g2)], axis=0)  # (4,P,R,E)
        ag = ag.reshape(4, P, 2, 2, E).transpose(1, 2, 0, 3, 4)  # (P, h, t, r', E)
        ag = np.ascontiguousarray(ag.reshape(P, 8 * H)).astype(f16)
        bb = np.stack([prep(b1), prep(b2)], axis=0)              # (2,P,R,E)
        bb = bb.reshape(2, P, 2, 2, E).transpose(1, 2, 0, 3, 4)  # (P, h, t, r', E)
        bb = np.ascontiguousarray(bb.reshape(P, 4 * H)).astype(f16)
        m4 = np.ascontiguousarray(prep(mf).reshape(P, W))
        in_maps.append({
            "a0": np.ascontiguousarray(ag[:, 0:H]),
            "a0b": np.ascontiguousarray(ag[:, H:2*H]),
            "g0w": np.ascontiguousarray(ag[:, 2*H:4*H]),
            "ag1": np.ascontiguousarray(ag[:, 4*H:]),
            "b4": bb,
            "m4": m4,
        })
    res = run_bass_kernel_spmd(nc, in_maps, core_ids=list(range(N_CORES)),
                               trace=_want_trace)
    out = np.concatenate([res.results[i]["out"] for i in range(N_CORES)])
    if _want_trace:
        kernel._last_result = res
    return out.astype(np.float32)


# revision 52
# speedup vs baseline: 1.0357x; 1.0357x over previous
"""Trainium2 Bass kernel for nn_AllAmplitude (helicity-amplitude intensity).

Math: the reference contracts two spin-1 Wigner-D matrices per (resonance,
event) with a Breit-Wigner weight and sums |amp|^2 over external helicities.
Because D1 @ D2 = D^1(U1 U2) for the SU(2) elements U1, U2 of the two
rotations, the whole intensity collapses to

  I = 7 sum_r |w_r|^2
    + sum_{r<r'} 2 Re(w_r conj(w_r')) (10 Re(av)^2 + 2 Im(av)^2 - 3)

with av = conj(a_r') a_r + b_r' conj(b_r), (a, b) the Cayley-Klein parameters
of the composed rotation, and w_r the complex Breit-Wigner weight.

v3 changes vs v2:
 - Host packs the 7 inputs into 4 wave buffers already in SBUF layout
   (AG0 = alpha/gamma halves r01, M, B4 = betas, AG1 = r23), giving
   4KB-contiguous per-partition DMA rows (vs 512B descriptors) and a
   deterministic arrival order matched to the compute schedule. All input
   DMAs issue serially from Sync; first DVE op starts ~5us earlier.
 - Single ACT table set (trig_and_small: sin/square/copy); float biases
   (const-AP pool) instead of a memset pi2 tile; the f32->fp16 reciprocal
   cast moved from DVE to an ACT copy.
 - DVE stream reordered to be gap-free: A(h0), A(h1), BW-denominator,
   B(h0), B(h1), wre/wim, pairs.  GpSimd does nothing (concurrent Pool
   ops poison DVE throughput ~4x on this silicon - measured).

Sharding: pure data parallelism over the event axis N=262144 across the 8
NeuronCores (32768 events each, laid out [128 partitions x 256 events] with
the R=4 resonance slices side by side in the free axis).
"""

import numpy as np

import concourse.bass as bass
from concourse import bacc, mybir
from concourse.bass_utils import run_bass_kernel_spmd

F32 = mybir.dt.float32
BF16 = mybir.dt.bfloat16
FP16 = mybir.dt.float16
I16 = mybir.dt.int16
ALU = mybir.AluOpType
ACTF = mybir.ActivationFunctionType

R = 4
N_TOTAL = 262144
N_CORES = 8
N_CORE = N_TOTAL // N_CORES     # 32768 events per core
P = 128                         # SBUF partitions
E = N_CORE // P                 # 256 events per partition per resonance
W = R * E                       # 1024 free-dim of a full working tile
H = W // 2                      # half tile (r01 / r23)

MAGIC = float(np.float32(1.5 * 2.0**23))   # round-to-nearest-int bias trick
INV4PI = float(np.float32(1.0 / (4.0 * np.pi)))
TWOPI = float(np.float32(2.0 * np.pi))
HALFPI = float(np.float32(np.pi / 2.0))


def _register_custom_ops():
    import concourse.dve_ops as dve_ops
    from concourse.dve_spec import Spec, Src0, Src1, sq, lower, _has_src1
    from concourse.dve_uop import DveOpSpec
    from concourse.dve_ops import DveOp

    if any(op.name == "ANT_RANGE_RED_ADD" for op in dve_ops.OPS):
        return {op.name: op for op in dve_ops.OPS}

    def make_op(name, spec):
        shas = {}
        for ver in ("v3", "v4"):
            uops = lower(spec, ver=ver)
            shas[ver] = DveOpSpec(name=name, opcode=31, uops=uops,
                                  rd1_en=_has_src1(spec)).sha(ver)
        return DveOp(name, spec, subdim=False, uops_sha=shas)

    def _rr_ref(sgn):
        def ref(in0, in1, s0, s1, imm2):
            t = ((in0 + sgn * in1) * s0 + s1).astype(np.float32)
            r = ((t + imm2).astype(np.float32) - imm2).astype(np.float32)
            return (t - r).astype(np.float32)
        return ref

    from concourse.dve_spec import C0, C1, C2
    u = (Src0 + Src1) * C0 + C1
    rr_add = make_op("ANT_RANGE_RED_ADD",
                     Spec(body=u - ((u + C2) - C2), reference=_rr_ref(1.0)))
    u2 = (Src0 - Src1) * C0 + C1
    rr_sub = make_op("ANT_RANGE_RED_SUB",
                     Spec(body=u2 - ((u2 + C2) - C2), reference=_rr_ref(-1.0)))
    chi = make_op("ANT_CHI", Spec(
        body=sq(Src0) * C0 + sq(Src1) * C1 + C2,
        reference=lambda in0, in1, s0, s1, imm2:
            (in0 * in0 * s0 + in1 * in1 * s1 + imm2).astype(np.float32)))
    den = make_op("ANT_DEN", Spec(
        body=sq(C0 - Src0) + C1,
        reference=lambda in0, in1, s0, s1, imm2:
            ((s0 - in0) * (s0 - in0) + s1).astype(np.float32)))

    for op in (rr_add, rr_sub, chi, den):
        dve_ops.OPS.append(op)
        dve_ops._SUB_OPCODE_FOR_NAME[op.name] = (
            dve_ops._CUSTOM_DVE_ROW_BASE + len(dve_ops.OPS) - 1)
        dve_ops.CUSTOM_DVE_SPECS[op.name] = op.spec
    assert max(dve_ops._SUB_OPCODE_FOR_NAME.values()) < 0x20
    return {op.name: op for op in dve_ops.OPS}


def _rs(r):
    return slice(r * E, (r + 1) * E)


def build(m0, g0, coef_r, coef_i):
    OPS = _register_custom_ops()
    RR_ADD, RR_SUB, CHI, DEN = (OPS["ANT_RANGE_RED_ADD"], OPS["ANT_RANGE_RED_SUB"],
                                OPS["ANT_CHI"], OPS["ANT_DEN"])
    AT = FP16

    nc = bacc.Bacc("TRN2", target_bir_lowering=False, debug=False,
                   num_devices=N_CORES)
    # packed input waves (host-side layout, see kernel())
    a0_in = nc.dram_tensor("a0", (P, 2 * H), FP16, kind="ExternalInput").ap()
    g0_in = nc.dram_tensor("g0w", (P, 2 * H), FP16, kind="ExternalInput").ap()
    m_in = nc.dram_tensor("m4", (P, W), F32, kind="ExternalInput").ap()
    b4_in = nc.dram_tensor("b4", (P, 4 * H), FP16, kind="ExternalInput").ap()
    ag1_in = nc.dram_tensor("ag1", (P, 4 * H), FP16, kind="ExternalInput").ap()
    out_ap = nc.dram_tensor("out", (N_CORE,), F32, kind="ExternalOutput").ap()

    f32 = np.float32
    m0 = m0.astype(np.float64); g0 = g0.astype(np.float64)
    cR = [float(f32(coef_r[r] * np.cos(coef_i[r]))) for r in range(R)]
    cI = [float(f32(coef_r[r] * np.sin(coef_i[r]))) for r in range(R)]
    m0sq = [float(f32(m0[r] * m0[r])) for r in range(R)]
    y = [float(f32(m0[r] * g0[r])) for r in range(R)]
    ysq = [float(f32(f32(y[r]) * f32(y[r]))) for r in range(R)]
    k1 = [float(f32(-f32(cI[r]) * f32(y[r]))) for r in range(R)]
    k2 = [float(f32(f32(cR[r]) * f32(y[r]))) for r in range(R)]
    c27 = [float(f32(7.0 * (f32(cR[r])**2 + f32(cI[r])**2))) for r in range(R)]

    # ---- static SBUF allocation ----
    alloc = []
    def sb(name, shape, dt=F32):
        t = nc.alloc_sbuf_tensor(name, list(shape), dt)
        alloc.append(t)
        return t.ap()

    AG = [sb("ag0t", [P, 4 * H], FP16), sb("ag1t", [P, 4 * H], FP16)]
    B4 = sb("b4t", [P, 4 * H], FP16)      # [b1h0|b2h0|b1h1|b2h1]
    M4 = sb("m4t", [P, W])                # f32 m
    pi2 = sb("pi2", [P, 1])
    dmy = sb("dmy", [P, 1], AT)
    UVWZ = sb("UVWZ", [P, 4 * W], AT)     # [ub|wb|vb|zb]
    ub = UVWZ[:, 0:W]; wb = UVWZ[:, W:2*W]
    vb = UVWZ[:, 2*W:3*W]; zb = UVWZ[:, 3*W:4*W]
    # packed fracs [fA|fB|fC|fD] and their absolute values, fp16
    f4 = sb("f4", [P, 4 * W], AT)
    a4 = sb("a4", [P, 4 * W], AT)
    fA = f4[:, 0:W]; fB = f4[:, W:2*W]; fC = f4[:, 2*W:3*W]; fD = f4[:, 3*W:4*W]
    # sin/cos of composite angles: [As|Bs|Cs|Ds|Ac|Bc|Cc|Dc]
    SCT8 = sb("SCT8", [P, 8 * W], AT)
    # beta trig packed [cb1|sb1|cb2|sb2]
    BP = sb("BP", [P, 4 * W], AT)
    # Wigner magnitudes [M0|M1|M3|M2] ( = [ms_ab | ms_cd] )
    MS4 = sb("MS4", [P, 4 * W], AT)
    # pq products [psab0|psab1|pscd0|pscd1|pcab0|pcab1|pccd0|pccd1]
    PQ8 = sb("PQ8", [P, 8 * W], AT)
    AB4 = sb("AB4", [P, 4 * W], AT)   # [are|bim|aim|bre]
    msq = sb("msq", [P, W]); den = sb("den", [P, W]); rcp = den
    wp1 = sb("wp1", [P, W], AT); wp2 = sb("wp2", [P, W], AT)
    rcph = sb("rcph", [P, W], AT)
    WH = sb("WH", [P, 2 * W], BF16)   # [wreh|wimh]
    wreh = WH[:, 0:W]; wimh = WH[:, W:2*W]
    dall = sb("dall", [P, W], AT); dh = sb("dh", [P, 2 * E], AT)
    dg = sb("dg", [P, E], AT)
    acc = sb("acc", [P, E])
    # pair scratch (reused across the 3 shift groups; DVE program order)
    NP3 = 3 * E
    PT = sb("PT", [P, 4 * NP3], AT)    # 4 packed products
    SD = sb("SD", [P, 4 * NP3], AT)    # [S1ch0|S1ch1|DTch0|DTch1]
    DT = SD[:, 2*NP3:4*NP3]
    QT = sb("QT", [P, 4 * NP3], AT)    # [q1|q4|q2|q3]
    NP6 = 6 * E
    AVI2 = sb("AVI2", [P, 2 * NP6], AT)   # [avr|avi]
    avr = AVI2[:, 0:NP6]; avi = AVI2[:, NP6:2*NP6]
    chis = sb("chis", [P, NP6], BF16); gw = sb("gw", [P, NP6], BF16)
    GT6 = sb("GT6", [P, 2 * NP6], BF16)
    term = sb("term", [P, NP6], BF16)

    sem_w = [nc.alloc_semaphore(f"s_w{i}") for i in range(6)]
    with (
        nc.semaphore("s_out") as sem_out,
        nc.semaphore("act_sem") as act_sem,
        nc.semaphore("vec_sem") as vec_sem,
        nc.semaphore("g_sem") as g_sem,
        nc.Block() as block,
    ):
        # ------------- GPSIMD: just the pi/2 bias tile ----------------------
        # (Pool DMA-issue is slower than Sync's and delays the first wave;
        #  Pool compute would poison concurrent DVE ops - keep it idle)
        @block.gpsimd
        def _(gpsimd):
            nc.gpsimd.memset(pi2[:], HALFPI).then_inc(g_sem, 1)
        # ------------- SYNC: all input DMA issue, first output half ---------
        @block.sync
        def _(sync):
            sync.dma_start(AG[0][:, 0:2*H], a0_in[:]).then_inc(sem_w[0], 16)
            sync.dma_start(AG[0][:, 2*H:4*H], g0_in[:]).then_inc(sem_w[1], 16)
            sync.dma_start(AG[1][:], ag1_in[:]).then_inc(sem_w[4], 16)
            sync.dma_start(M4[:], m_in[:]).then_inc(sem_w[2], 16)
            sync.dma_start(B4[:], b4_in[:]).then_inc(sem_w[3], 16)
            # (issue order = HBM service order; m before b4 so msq lands
            #  well ahead of the DVE den stage at either clock state)
            outv = out_ap.rearrange("(p e) -> p e", p=P, e=E)
            sync.wait_ge(vec_sem, 8)
            sync.dma_start(outv[:, 0:E//2], acc[:, 0:E//2]).then_inc(sem_out, 16)
            sync.wait_ge(sem_out, 32)

        # ------------- SCALAR (ACT) ----------------------------------------
        # act_sem: 1 dummy (forces the trig table load early), 2 msq,
        #          h0 sins: 3 cbs, 4 sbs, 5 AsBs, 6 CsDs, 7 cos4,
        #          h1 sins: 8-12, 13 rcph, 14-17 wp1, 18-21 wp2, 22-25 dall
        @block.scalar
        def _(scalar):
            bpv = BP.rearrange("p (c w) -> p c w", c=4, w=W)       # cb1 sb1 cb2 sb2
            b4v = B4.rearrange("p (c w) -> p c w", c=4, w=H)       # b1h0 b2h0 b1h1 b2h1
            sct8v = SCT8.rearrange("p (c w) -> p c w", c=8, w=W)
            f4v = f4.rearrange("p (c w) -> p c w", c=4, w=W)
            a4v = a4.rearrange("p (c w) -> p c w", c=4, w=W)
            scalar.wait_ge(g_sem, 1)
            # dummy 1-element Sin: hoists the trig ACT-table load to ~6us,
            # before any input data arrives (Square/Copy share this table)
            scalar.activation(dmy[:], pi2[:], ACTF.Sin).then_inc(act_sem, 1)  # 1
            scalar.wait_ge(sem_w[2], 16)
            scalar.activation(msq[:], M4[:], ACTF.Square).then_inc(act_sem, 1)  # 2
            scalar.wait_ge(sem_w[3], 16)
            for h in range(2):
                s = slice(h * H, h * H + H)
                # cos(b/2), sin(b/2) for both chains, this half
                scalar.activation(bpv[:, 0::2, s], b4v[:, 2*h:2*h+2, :],
                                  ACTF.Sin, scale=0.5,
                                  bias=pi2[:]).then_inc(act_sem, 1)  # cbs
                scalar.activation(bpv[:, 1::2, s], b4v[:, 2*h:2*h+2, :],
                                  ACTF.Sin, scale=0.5).then_inc(act_sem, 1)  # sbs
                scalar.wait_ge(vec_sem, 1 + 3 * h)    # fA,fB half h
                scalar.activation(sct8v[:, 0:2, s], f4v[:, 0:2, s], ACTF.Sin,
                                  scale=TWOPI).then_inc(act_sem, 1)   # AsBs
                scalar.wait_ge(vec_sem, 2 + 3 * h)    # fC,fD half h
                scalar.activation(sct8v[:, 2:4, s], f4v[:, 2:4, s], ACTF.Sin,
                                  scale=-TWOPI).then_inc(act_sem, 1)  # CsDs
                scalar.wait_ge(vec_sem, 3 + 3 * h)    # abs half h (all four)
                scalar.activation(sct8v[:, 4:8, s], a4v[:, :, s], ACTF.Sin,
                                  scale=-TWOPI, bias=pi2[:]).then_inc(act_sem, 1)
            scalar.wait_ge(vec_sem, 7)   # rcp
            scalar.activation(rcph[:], rcp[:], ACTF.Copy).then_inc(act_sem, 1)  # 13
            for r in range(R):
                scalar.activation(wp1[:, _rs(r)], msq[:, _rs(r)], ACTF.Copy,
                                  scale=-cR[r],
                                  bias=float(f32(cR[r]*m0sq[r] + k1[r]))
                                  ).then_inc(act_sem, 1)               # 14..17
            for r in range(R):
                scalar.activation(wp2[:, _rs(r)], msq[:, _rs(r)], ACTF.Copy,
                                  scale=-cI[r],
                                  bias=float(f32(cI[r]*m0sq[r] + k2[r]))
                                  ).then_inc(act_sem, 1)               # 18..21
            for r in range(R):
                scalar.activation(dall[:, _rs(r)], rcph[:, _rs(r)], ACTF.Copy,
                                  scale=c27[r]).then_inc(act_sem, 1)   # 22..25
            outv2 = out_ap.rearrange("(p e) -> p e", p=P, e=E)
            scalar.wait_ge(vec_sem, 9)
            scalar.dma_start(outv2[:, E//2:], acc[:, E//2:]).then_inc(sem_out, 16)

        # ------------- VECTOR (DVE) -----------------------------------------
        # vec_sem: per half h: 1+3h fAB, 2+3h fCD, 3+3h abs; 7 rcp;
        #          8/9 acc halves
        @block.vector
        def _(vector):
            # ---- stage A: per-half pre-adds + RR + packed abs ----
            agv = [AG[h].rearrange("p (c w) -> p c w", c=4, w=H) for h in range(2)]
            f4i = f4.bitcast(I16).rearrange("p (c w) -> p c w", c=4, w=W)
            a4i = a4.bitcast(I16).rearrange("p (c w) -> p c w", c=4, w=W)
            uvwz_v = UVWZ.rearrange("p (c w) -> p c w", c=4, w=W)
            for h in range(2):
                s = slice(h * H, h * H + H)
                if h == 0:
                    vector.wait_ge(sem_w[0], 16)   # alphas h0
                    nc.vector.tensor_add(ub[:, s], agv[0][:, 0, :], agv[0][:, 1, :])
                    nc.vector.tensor_sub(vb[:, s], agv[0][:, 0, :], agv[0][:, 1, :])
                    vector.wait_ge(sem_w[1], 16)   # gammas h0
                    nc.vector.tensor_add(wb[:, s], agv[0][:, 2, :], agv[0][:, 3, :])
                    nc.vector.tensor_sub(zb[:, s], agv[0][:, 2, :], agv[0][:, 3, :])
                else:
                    vector.wait_ge(sem_w[4], 16)   # AG1
                    # [ub|wb] = [a1|g1] + [a2|g2]; [vb|zb] = [a1|g1] - [a2|g2]
                    nc.vector.tensor_add(uvwz_v[:, 0:2, s], agv[1][:, 0::2, :],
                                         agv[1][:, 1::2, :])
                    nc.vector.tensor_sub(uvwz_v[:, 2:4, s], agv[1][:, 0::2, :],
                                         agv[1][:, 1::2, :])
                nc.vector._custom_dve(RR_ADD, out=fA[:, s], in0=ub[:, s],
                                      in1=wb[:, s], s0=INV4PI, s1=0.0, imm2=MAGIC)
                nc.vector._custom_dve(RR_SUB, out=fB[:, s], in0=vb[:, s],
                                      in1=zb[:, s], s0=INV4PI, s1=0.0,
                                      imm2=MAGIC).then_inc(vec_sem, 1)  # 1/4
                nc.vector._custom_dve(RR_ADD, out=fC[:, s], in0=ub[:, s],
                                      in1=zb[:, s], s0=INV4PI, s1=0.0, imm2=MAGIC)
                nc.vector._custom_dve(RR_SUB, out=fD[:, s], in0=vb[:, s],
                                      in1=wb[:, s], s0=INV4PI, s1=0.0,
                                      imm2=MAGIC).then_inc(vec_sem, 1)  # 2/5
                nc.vector.tensor_scalar(a4i[:, :, s], f4i[:, :, s], 0x7FFF, None,
                                        ALU.bitwise_and).then_inc(vec_sem, 1)  # 3/6

            # ---- Breit-Wigner denominator (fills the pre-B ACT window) ----
            vector.wait_ge(act_sem, 2)   # msq
            for r in range(R):
                nc.vector._custom_dve(DEN, out=den[:, _rs(r)], in0=msq[:, _rs(r)],
                                      s0=m0sq[r], s1=ysq[r])
            nc.vector.reciprocal_approx_fast(out=rcp[:], in_=den[:]) \
                .then_inc(vec_sem, 1)   # 7

            # ---- stage B: Wigner magnitudes + merged 4-ch pq products ----
            ms4v = MS4.rearrange("p (c w) -> p c w", c=4, w=W)
            ms_ab = MS4[:, 0:2*W].rearrange("p (c w) -> p c w", c=2, w=W)
            ms_cd = MS4[:, 2*W:4*W].rearrange("p (c w) -> p c w", c=2, w=W)
            b1v = BP[:, 0:2*W].rearrange("p (c w) -> p c w", c=2, w=W)
            b2v = BP[:, 2*W:4*W].rearrange("p (c w) -> p c w", c=2, w=W)
            sct8vv = SCT8.rearrange("p (c w) -> p c w", c=8, w=W)
            pq8v = PQ8.rearrange("p (c w) -> p c w", c=8, w=W)
            ab4v = AB4.rearrange("p (c w) -> p c w", c=4, w=W)
            for h in range(2):
                s = slice(h * H, h * H + H)
                vector.wait_ge(act_sem, 4 + 5 * h)   # cbs+sbs half h
                nc.vector.tensor_mul(ms_ab[:, :, s], b1v[:, :, s], b2v[:, :, s])
                nc.vector.tensor_mul(ms_cd[:, :, s], b1v[:, :, s], b2v[:, 1::-1, s])
                vector.wait_ge(act_sem, 6 + 5 * h)   # AsBs+CsDs half h
                nc.vector.tensor_mul(pq8v[:, 0:4, s], ms4v[:, :, s],
                                     sct8vv[:, 0:4, s])
                vector.wait_ge(act_sem, 7 + 5 * h)   # cos4 half h
                nc.vector.tensor_mul(pq8v[:, 4:8, s], ms4v[:, :, s],
                                     sct8vv[:, 4:8, s])
                # AB4 = [are|bim|aim|bre]
                # [are|aim] = [Ac-prod - Bc-prod | Bs-prod - As-prod]
                nc.vector.tensor_sub(ab4v[:, 0::2, s], pq8v[:, 4::-3, s],
                                     pq8v[:, 5::-5, s])
                # [bim|bre] = [Cs-prod + Ds-prod | Cc-prod + Dc-prod]
                nc.vector.tensor_add(ab4v[:, 1::2, s], pq8v[:, 2::4, s],
                                     pq8v[:, 3::4, s])

            # ---- stage D: pairs (angle part; independent of the BW chain) ----
            gt6v = GT6.rearrange("p (c w) -> p c w", c=2, w=NP6)
            whv = WH.rearrange("p (c w) -> p c w", c=2, w=W)
            goffs = {1: 0, 2: 3 * E, 3: 5 * E}
            for sig in (1, 2, 3):
                n = (R - sig) * E
                go = goffs[sig]
                L = slice(0, n)
                Rr = slice(sig * E, sig * E + n)
                ptv = PT.rearrange("p (c w) -> p c w", c=4, w=NP3)
                sd4v = SD.rearrange("p (c w) -> p c w", c=4, w=NP3)
                av2v = AVI2.rearrange("p (c w) -> p c w", c=2, w=NP6)
                # ptv channels: [p_are|p_bim|p_aim|p_bre]
                nc.vector.tensor_mul(ptv[:, :, :n], ab4v[:, :, L], ab4v[:, :, Rr])
                # sd = [p_are+p_bre | p_aim+p_bim]
                nc.vector.tensor_add(sd4v[:, 0:2, :n],
                                     ptv[:, 0::2, :n], ptv[:, 3::-2, :n])
                qtv = QT.rearrange("p (c w) -> p c w", c=4, w=NP3)
                # one 4-ch op: [q1|q4|q2|q3] = [are|bim|aim|bre][L] *
                #              [aim|bre|are|bim][R] (group-reversed view)
                abgv = AB4.rearrange("p (g c w) -> p g c w", g=2, c=2, w=W)
                nc.vector.tensor_mul(
                    qtv[:, :, :n].rearrange("p (g c) w -> p g c w", g=2, c=2),
                    abgv[:, :, :, L], abgv[:, 1::-1, :, Rr])
                nc.vector.tensor_sub(sd4v[:, 2:4, :n],
                                     qtv[:, 0:2, :n], qtv[:, 2:4, :n])
                # [avr|avi] in one 2-ch add: ch0 = S1c0+S1c1, ch1 = DTc0+DTc1
                nc.vector.tensor_add(av2v[:, :, go:go+n],
                                     sd4v[:, 0::2, :n], sd4v[:, 1::2, :n])

            # ---- BW weights + pair weight products (late: ACT has slack) ----
            vector.wait_ge(act_sem, 21)   # wp1/wp2 + rcph done
            nc.vector.tensor_mul(wreh[:], wp1[:], rcph[:])
            nc.vector.tensor_mul(wimh[:], wp2[:], rcph[:])
            for sig in (1, 2, 3):
                n = (R - sig) * E
                go = goffs[sig]
                L = slice(0, n)
                Rr = slice(sig * E, sig * E + n)
                nc.vector.tensor_mul(gt6v[:, :, go:go+n], whv[:, :, L], whv[:, :, Rr])
            # diagonal (off the critical tail: only needs dall)
            vector.wait_ge(act_sem, 25)   # dall
            nc.vector.tensor_add(dh[:], dall[:, 0:2*E], dall[:, 2*E:4*E])
            nc.vector.tensor_add(dg[:], dh[:, 0:E], dh[:, E:2*E])
            nc.vector._custom_dve(CHI, out=chis[:], in0=avr[:], in1=avi[:],
                                  s0=20.0, s1=4.0, imm2=-6.0)
            nc.vector.tensor_add(gw[:], GT6[:, 0:NP6], GT6[:, NP6:2*NP6])
            nc.vector.tensor_mul(term[:], chis[:], gw[:])
            # tree-sum the 6 pair blocks: one packed 3-pair add, then merge
            tv = term.rearrange("p (c w) -> p c w", c=6, w=E)
            s3 = DT.rearrange("p (c w) -> p c w", c=2, w=NP3)  # reuse DT scratch
            nc.vector.tensor_tensor(s3[:, 0, 0:3*E].rearrange("p (c w) -> p c w", c=3, w=E),
                                    tv[:, 0::2, :], tv[:, 1::2, :], ALU.add)
            nc.vector.tensor_add(DT[:, 3*E:4*E], DT[:, 0:E], DT[:, E:2*E])
            nc.vector.tensor_add(dh[:, 0:E], DT[:, 2*E:3*E], DT[:, 3*E:4*E])
            # final: acc = pairs + diagonal, split for output DMA overlap
            nc.vector.tensor_add(acc[:, 0:E//2], dh[:, 0:E//2], dg[:, 0:E//2]) \
                .then_inc(vec_sem, 1)   # 8
            nc.vector.tensor_add(acc[:, E//2:E], dh[:, E//2:E], dg[:, E//2:E]) \
                .then_inc(vec_sem, 1)   # 9

    nc.compile()
    return nc


_CACHE = {}


def kernel(alpha1, beta1, gamma1, alpha2, beta2, gamma2, m, m0, g0,
           coef_r, coef_i, _want_trace=False):
    key = (np.asarray(m0, np.float32).tobytes(), np.asarray(g0, np.float32).tobytes(),
           np.asarray(coef_r, np.float32).tobytes(), np.asarray(coef_i, np.float32).tobytes())
    if key not in _CACHE:
        _CACHE[key] = build(np.asarray(m0, np.float32), np.asarray(g0, np.float32),
                            np.asarray(coef_r, np.float32), np.asarray(coef_i, np.float32))
    nc = _CACHE[key]

    f16 = np.float16
    a1 = np.asarray(alpha1, np.float32); a2 = np.asarray(alpha2, np.float32)
    g1 = np.asarray(gamma1, np.float32); g2 = np.asarray(gamma2, np.float32)
    b1 = np.asarray(beta1, np.float32); b2 = np.asarray(beta2, np.float32)
    mf = np.asarray(m, np.float32)

    in_maps = []
    for i in range(N_CORES):
        sl = slice(i * N_CORE, (i + 1) * N_CORE)
        # (R, N_CORE) -> (P, R, E): partition-major event blocks
        def prep(x):
            return x[:, sl].reshape(R, P, E).transpose(1, 0, 2)
        ag = np.stack([prep(a1), prep(a2), prep(g1), prep(g2)], axis=0)  # (4,P,R,E)
        ag = ag.reshape(4, P, 2, 2, E).transpose(1, 2, 0, 3, 4)  # (P, h, t, r', E)
        ag = np.ascontiguousarray(ag.reshape(P, 8 * H)).astype(f16)
        bb = np.stack([prep(b1), prep(b2)], axis=0)              # (2,P,R,E)
        bb = bb.reshape(2, P, 2, 2, E).transpose(1, 2, 0, 3, 4)  # (P, h, t, r', E)
        bb = np.ascontiguousarray(bb.reshape(P, 4 * H)).astype(f16)
        m4 = np.ascontiguousarray(prep(mf).reshape(P, W))
        in_maps.append({
            "a0": np.ascontiguousarray(ag[:, 0:2*H]),
            "g0w": np.ascontiguousarray(ag[:, 2*H:4*H]),
            "ag1": np.ascontiguousarray(ag[:, 4*H:]),
            "b4": bb,
            "m4": m4,
        })
    res = run_bass_kernel_spmd(nc, in_maps, core_ids=list(range(N_CORES)),
                               trace=_want_trace)
    out = np.concatenate([res.results[i]["out"] for i in range(N_CORES)])
    if _want_trace:
        kernel._last_result = res
    return out.astype(np.float32)
